# revision 23
# baseline (speedup 1.0000x reference)
"""GAT-mod forward on 8 trn2 NeuronCores (Bass/Tile).

Strategy (dst-sharded, x1-space aggregation):
- Nodes are partitioned across 8 cores by destination id (6250 each).
- Key identity: h = W_gat @ x1 is linear, so the GAT aggregation
  out[n,h,:] = sum_e alpha_e * h[src_e,h,:] = W_h @ (sum_e alpha_e x1[src_e]).
  We aggregate in x1-space (64 wide per head-weight, 4 heads share the same
  x1) and apply W_gat per 128-node window AFTER normalization.
- Each core builds the full node table T[n] = [x1 bf16(64) | a_src f32(4) | pad]
  (256B rows, the dma_gather minimum) in its local HBM, split at row 25000 so
  gather indices fit int16, with a PAD row per half (x1=0, a_src=-1e4).
- Edges (incl. self-loops) are grouped by 128-node destination windows, split
  into lo/hi source halves, packed into 128-slot batches; 16 batches per
  gather call (2048 idxs). Gathers use prepare_only+trigger_dma so gpsimd only
  pays descriptor generation; transfers run async on the DMA queues.
  Per batch: e = lrelu(a_src + IndT@a_dst); p = exp(e) (no-max softmax, e is
  bounded); msg = [x1*p per head | p] (260 wide); PSUM[node,260] += Ind^T@msg.
- Per window finalize: z/denom -> 2 PE transposes -> 2 matmuls with stacked
  W_gat^T -> y = 0.25*sum_h + bias; BN stats via per-window PE matmuls.
- BN batch stats via AllReduce across the 8 cores; bulk BN apply + store.
"""

import os
import sys
import hashlib

import numpy as np
import ml_dtypes

N = 50000
E = 800000
D = 64
H = 4
NEG = 0.2
BN_EPS = 1e-5
NC = 8
SLAB = N // NC          # 6250
W = 128                 # window node slots
NW = 53                 # windows per core (edge-balanced, ~118 nodes each)
LO = 25000
RE = 128                # table row elems (bf16): 64 x1 + 8 (4 f32 a_src) + pad
TROWS = 2 * LO + 2      # 50002 (two pad rows)
PAD_LO = LO             # pad row index within lo half
PAD_HI = LO             # within hi half (row 25001+25000 = 50001)
BPC = 8                 # batches per gather call
CALL = BPC * 128        # 1024 idxs per gather (HW ucode caps at 1024)

_CACHE = {}
LAST_EXEC_NS = None
LAST_TRACE = None


def _install_ntff_shim():
    import contextlib
    import ctypes
    import types

    if "antenv.axon_hooks" in sys.modules:
        return
    so_path = "/opt/axon/libaxon_pjrt.so"

    def _hook_factory(so_path):
        try:
            lib = ctypes.CDLL(so_path)
        except OSError:
            return None
        if not hasattr(lib, "axon_start_nrt_profile"):
            return None
        lib.axon_start_nrt_profile.argtypes = [ctypes.POINTER(ctypes.c_int64), ctypes.c_size_t]
        lib.axon_start_nrt_profile.restype = ctypes.c_int64
        lib.axon_stop_nrt_profile.argtypes = [ctypes.c_char_p]
        lib.axon_stop_nrt_profile.restype = ctypes.c_int64

        @contextlib.contextmanager
        def _hook(output_dir, device_ids):
            import jax

            jax.devices()
            if device_ids:
                ids = (ctypes.c_int64 * len(device_ids))(*device_ids)
                rc = lib.axon_start_nrt_profile(ids, len(device_ids))
            else:
                rc = lib.axon_start_nrt_profile(None, 0)
            if rc != 0:
                raise RuntimeError(f"axon_start_nrt_profile rc={rc}")
            try:
                yield
            finally:
                lib.axon_stop_nrt_profile(str(output_dir).encode())

        return _hook

    mod = types.ModuleType("antenv.axon_hooks")
    _h = [None]
    mod.set_axon_ntff_profile_hook = lambda h: _h.__setitem__(0, h)
    mod.get_axon_ntff_profile_hook = lambda: _h[0]
    sys.modules["antenv.axon_hooks"] = mod
    try:
        import antenv

        antenv.axon_hooks = mod
    except ImportError:
        pass
    mod.set_axon_ntff_profile_hook(_hook_factory(so_path))


# ----------------------------------------------------------------- host prep
def _node_permutation(edge_index):
    """Greedy edge-balanced packing of nodes into (window, core) buckets:
    minimizes the max per-(window, src-half) edge count so every section
    fits its batch budget on every core."""
    src = np.concatenate([edge_index[0].astype(np.int64), np.arange(N, dtype=np.int64)])
    dst = np.concatenate([edge_index[1].astype(np.int64), np.arange(N, dtype=np.int64)])
    d_lo = np.bincount(dst[src < LO], minlength=N).astype(np.int64)
    d_hi = np.bincount(dst[src >= LO], minlength=N).astype(np.int64)
    order = np.argsort(-(d_lo + d_hi), kind="stable")
    nbk = NW * NC
    fill_n = np.zeros(nbk, np.int64)
    fill_l = np.zeros(nbk, np.float64)
    fill_h = np.zeros(nbk, np.float64)
    bucket_core = np.tile(np.arange(NC), NW)
    perm_core = np.empty(N, np.int64)
    perm_slot = np.empty(N, np.int64)
    for n in order:
        score = np.maximum(fill_l + d_lo[n], fill_h + d_hi[n])
        score = np.where(fill_n >= 128, 1e18, score)
        b = int(np.argmin(score))
        perm_core[n] = bucket_core[b]
        perm_slot[n] = (b // NC) * 128 + fill_n[b]
        fill_n[b] += 1
        fill_l[b] += d_lo[n]
        fill_h[b] += d_hi[n]
    return perm_core, perm_slot


def _schedule_and_blobs(edge_index):
    src = np.concatenate([edge_index[0].astype(np.int64), np.arange(N, dtype=np.int64)])
    dst = np.concatenate([edge_index[1].astype(np.int64), np.arange(N, dtype=np.int64)])
    perm_core, perm_slot = _node_permutation(edge_index)

    cores = []
    for c in range(NC):
        sel = perm_core[dst] == c
        s_src = src[sel]
        s_dst = perm_slot[dst[sel]]
        islo = s_src < LO
        win = s_dst >> 7
        secid = win * 2 + (1 - islo.astype(np.int64))  # even = lo, odd = hi
        order = np.argsort(secid, kind="stable")
        cores.append((s_src[order], s_dst[order], secid[order]))

    # per-(core, section) counts; shared schedule = max over cores
    NSEC = NW * 2
    cnts = np.zeros((NC, NSEC), np.int64)
    for c in range(NC):
        binc = np.bincount(cores[c][2], minlength=NSEC)
        cnts[c] = binc
    nb_sec = (np.max(cnts, axis=0) + 127) // 128  # batches per section
    nb_sec = np.maximum(nb_sec, 1)

    # batch list: lo run (even sections, w ascending), then hi run
    batches = []  # (w, kind, sec, dead)
    for kind in (0, 1):  # 0=lo, 1=hi
        run_start = len(batches)
        for wdx in range(NW):
            s = wdx * 2 + kind
            for _ in range(int(nb_sec[s])):
                batches.append([wdx, kind, s, False])
        while (len(batches) - run_start) % BPC != 0:
            batches.append([0, kind, -1, True])
    NB = len(batches)
    NCALLS = NB // BPC

    # within-call permutation: move each section's (globally) final batch and
    # dead batches to the back of their call so their pad slots form a
    # trailing run that the gather ucode's negative-index trim can skip.
    sec_last_global = {}
    for bi, (wdx, kind, s, dead) in enumerate(batches):
        if not dead:
            sec_last_global[s] = bi
    order = []
    for ci in range(NCALLS):
        idxs = list(range(ci * BPC, (ci + 1) * BPC))
        front = [b for b in idxs
                 if not batches[b][3] and sec_last_global[batches[b][2]] != b]
        back = [b for b in idxs
                if batches[b][3] or sec_last_global[batches[b][2]] == b]
        order += front + back
    batches = [batches[b] for b in order]
    call_kind = [batches[ci * BPC][1] for ci in range(NCALLS)]

    # mark section start/stop per batch (execution order = list order)
    sec_first = {}
    sec_last = {}
    for bi, (wdx, kind, s, dead) in enumerate(batches):
        if dead:
            continue
        if s not in sec_first:
            sec_first[s] = bi
        sec_last[s] = bi
    binfo = []
    for bi, (wdx, kind, s, dead) in enumerate(batches):
        binfo.append(dict(w=wdx, kind=kind, sec=s, dead=dead,
                          start=(not dead and sec_first[s] == bi),
                          stop=(not dead and sec_last[s] == bi)))

    sched = dict(NB=NB, NCALLS=NCALLS, call_kind=call_kind, binfo=binfo)

    # per-core blobs
    blobs = []
    for c in range(NC):
        s_src, s_dst, s_sec = cores[c]
        gidx = np.full((NB * 128,), PAD_LO, np.int64)
        filled = np.zeros((NB * 128,), bool)
        ind = np.zeros((NB, 128, 128), np.float32)
        indt = np.zeros((NB, 128, 128), np.float32)
        # per-section edge ranges; edges fill each section's batches in
        # execution order
        sec_edge_start = np.zeros(NSEC + 1, np.int64)
        np.cumsum(np.bincount(s_sec, minlength=NSEC), out=sec_edge_start[1:])
        sec_ptr = sec_edge_start[:-1].copy()
        for bi, info in enumerate(binfo):
            if info["dead"]:
                continue
            s = info["sec"]
            e0 = int(sec_ptr[s])
            e1 = min(e0 + 128, int(sec_edge_start[s + 1]))
            n = e1 - e0
            sec_ptr[s] = e1
            if n <= 0:
                continue
            srcs = s_src[e0:e1]
            idxv = srcs if info["kind"] == 0 else srcs - LO
            gidx[bi * 128:bi * 128 + n] = idxv
            filled[bi * 128:bi * 128 + n] = True
            node_in_w = (s_dst[e0:e1] - info["w"] * 128).astype(np.int64)
            sloc = np.arange(n)
            ind[bi, sloc, node_in_w] = 1.0
            indt[bi, node_in_w, sloc] = 1.0
        # trailing pads of each call become -1 (ucode trims them); keep the
        # first 8 calls on the PAD row since their gt buffers start
        # uninitialized (later calls see finite stale data, safe).
        if os.environ.get("GAT_NOTRIM", "0") != "1":
            for ci in range(8, NCALLS):
                blk = filled[ci * CALL:(ci + 1) * CALL]
                nz = np.nonzero(blk)[0]
                last = nz[-1] if len(nz) else -1
                gidx[ci * CALL + last + 1:(ci + 1) * CALL] = -1
        # wrap gather indices: call ci covers positions [ci*CALL, +CALL)
        g16 = gidx.astype(np.int16).reshape(NCALLS, CALL // 16, 16)
        gw = np.transpose(g16, (0, 2, 1)).reshape(NCALLS, 16, CALL // 16)
        gw = np.tile(gw, (1, 8, 1))  # [NCALLS, 128, CALL//16]
        GIDX = np.ascontiguousarray(
            np.transpose(gw, (1, 0, 2)).reshape(128, NCALLS * (CALL // 16)))
        both = np.concatenate([ind.reshape(NCALLS, BPC, 128, 128),
                               indt.reshape(NCALLS, BPC, 128, 128)], axis=1)
        INDB = np.ascontiguousarray(
            np.transpose(both, (2, 0, 1, 3)).reshape(128, NB * 256)).astype(ml_dtypes.float8_e4m3)
        blobs.append(dict(GIDX=GIDX, INDB=INDB))
    sched["perm_core"] = perm_core
    sched["perm_slot"] = perm_slot
    return sched, blobs


def _build_program(sched, prelu_uniform=True, prelu_value=0.25):
    from concourse import bacc, masks, mybir
    from concourse.tile import TileContext

    AL = mybir.AluOpType
    AF = mybir.ActivationFunctionType
    f32 = mybir.dt.float32
    bf16 = mybir.dt.bfloat16
    fp8 = mybir.dt.float8e4
    i16 = mybir.dt.int16

    NB = sched["NB"]
    NCALLS = sched["NCALLS"]
    binfo = sched["binfo"]

    nc = bacc.Bacc("TRN2", target_bir_lowering=False, debug=False,
                   num_devices=NC, num_swdge_queues=4)

    xT = nc.dram_tensor("xT", (D, N), bf16, kind="ExternalInput")
    xTs = nc.dram_tensor("xTs", (D, NW * 128), bf16, kind="ExternalInput")
    W1B = nc.dram_tensor("W1B", (128, 128), bf16, kind="ExternalInput")
    W1T = nc.dram_tensor("W1T", (D, D), bf16, kind="ExternalInput")
    CS2 = nc.dram_tensor("CS2", (128, 8), bf16, kind="ExternalInput")
    CD = nc.dram_tensor("CD", (D, 4), bf16, kind="ExternalInput")
    WSTK = nc.dram_tensor("WSTK", (128, 2 * D), bf16, kind="ExternalInput")
    prelu2 = nc.dram_tensor("prelu2", (128, 1), f32, kind="ExternalInput")
    prelu1 = nc.dram_tensor("prelu1", (D, 1), f32, kind="ExternalInput")
    GIDX = nc.dram_tensor("GIDX", (128, NCALLS * (CALL // 16)), i16, kind="ExternalInput")
    INDB = nc.dram_tensor("INDB", (128, NB * 256), fp8, kind="ExternalInput")
    bias128 = nc.dram_tensor("bias128", (128, D), f32, kind="ExternalInput")
    MASKS = nc.dram_tensor("MASKS", (128, NW), f32, kind="ExternalInput")
    gb_row = nc.dram_tensor("gb_row", (1, 128), f32, kind="ExternalInput")  # [gamma|beta]
    out_slab = nc.dram_tensor("out_slab", (NW * 128, D), f32, kind="ExternalOutput")

    with TileContext(nc) as tc:
        with tc.tile_pool(name="dram", bufs=1, space="DRAM") as dpool, \
             tc.tile_pool(name="dram_lo", bufs=1, space="DRAM") as dpool_lo, \
             tc.tile_pool(name="dram_hi", bufs=1, space="DRAM") as dpool_hi, \
             tc.tile_pool(name="persist", bufs=1) as pp:
            table_lo = dpool_lo.tile([LO + 1, RE], bf16)
            table_hi = dpool_hi.tile([LO + 1, RE], bf16)
            cc_in = dpool.tile([1, 128], f32)
            cc_out = dpool.tile([1, 128], f32)

            w1b_sb = pp.tile([128, 128], bf16)
            nc.sync.dma_start(w1b_sb[:], W1B[:, :])
            w1t_sb = pp.tile([D, D], bf16)
            nc.sync.dma_start(w1t_sb[:], W1T[:, :])
            cs2_sb = pp.tile([128, 8], bf16)
            nc.sync.dma_start(cs2_sb[:], CS2[:, :])
            cd_sb = pp.tile([D, 4], bf16)
            nc.sync.dma_start(cd_sb[:], CD[:, :])
            wstk_sb = pp.tile([128, 2, D], bf16)
            nc.sync.dma_start(wstk_sb[:].rearrange("p a b -> p (a b)"), WSTK[:, :])
            prelu2_sb = pp.tile([128, 1], f32)
            nc.sync.dma_start(prelu2_sb[:], prelu2[:, :])
            prelu1_sb = pp.tile([D, 1], f32)
            nc.sync.dma_start(prelu1_sb[:], prelu1[:, :])
            bias_sb = pp.tile([128, D], f32)
            nc.sync.dma_start(bias_sb[:], bias128[:, :])
            masks_sb = pp.tile([128, NW], f32)
            nc.sync.dma_start(masks_sb[:], MASKS[:, :])
            gb_sb = pp.tile([1, 128], f32)
            nc.sync.dma_start(gb_sb[:], gb_row[:, :])
            gidx_sb = pp.tile([128, NCALLS * (CALL // 16)], i16)
            nc.sync.dma_start(gidx_sb[:], GIDX[:, :])
            ident = pp.tile([128, 128], bf16)
            masks.make_identity(nc, ident[:])
            e30 = pp.tile([128, 1], f32)
            nc.vector.memset(e30[:], 1e-30)
            ebn = pp.tile([1, 1], f32)
            nc.vector.memset(ebn[:], BN_EPS)

            a_dst = pp.tile([128, NW, 4], bf16)
            slab = pp.tile([128, NW, 260], f32)
            y_sb = pp.tile([128, NW, D], f32)

            # -------- phase A + lo table, then lo-gathers ‖ hi table, then hi ----
            with tc.tile_pool(name="pt_sb", bufs=3) as tp, \
                 tc.tile_pool(name="pe_g", bufs=8) as gp, \
                 tc.tile_pool(name="pe_i", bufs=4) as ip, \
                 tc.tile_pool(name="pe_s", bufs=3) as sp, \
                 tc.tile_pool(name="pe_m", bufs=5) as mp, \
                 tc.tile_pool(name="pe_z", bufs=1) as zp, \
                 tc.tile_pool(name="pe_wp", bufs=2, space="PSUM") as wp, \
                 tc.tile_pool(name="pe_ap", bufs=2, space="PSUM") as app:

                def prelu_apply(x1_out, m1_in, np_, nw_, tag):
                    # m1_in is PSUM f32; copy to bf16 SBUF on Scalar, then one
                    # DVE op computes max(w*m, m) on the 16-bit copy.
                    m1c = tp.tile([128, 512], bf16, tag=tag)
                    nc.scalar.copy(m1c[:np_, :nw_], m1_in)
                    if prelu_uniform:
                        nc.vector.scalar_tensor_tensor(
                            out=x1_out, in0=m1c[:np_, :nw_], scalar=prelu_value,
                            in1=m1c[:np_, :nw_], op0=AL.mult, op1=AL.max)
                    else:
                        pw = prelu2_sb if np_ == 128 else prelu1_sb
                        tmp = tp.tile([128, 512], bf16, tag=tag + "w")
                        nc.vector.tensor_tensor(
                            out=tmp[:np_, :nw_], in0=m1c[:np_, :nw_],
                            in1=pw[:, :].broadcast_to([np_, nw_]),
                            op=AL.mult)
                        nc.vector.tensor_tensor(out=x1_out, in0=tmp[:np_, :nw_],
                                                in1=m1c[:np_, :nw_], op=AL.max)

                def store_span(r0, nrows, src_ap):
                    # store src_ap [128, nj, RE] (row r = r0 + j*128 + p) to the
                    # split tables; nrows a multiple of 128 except final block.
                    nj = (nrows + 127) // 128
                    if r0 + nrows <= LO or r0 >= LO:
                        tbl = table_lo if r0 + nrows <= LO else table_hi
                        rb = r0 if r0 + nrows <= LO else r0 - LO
                        if nrows % 128 == 0:
                            dstp = tbl[rb:rb + nrows, :].rearrange(
                                "(j p) e -> p j e", p=128)
                            nc.scalar.dma_start(dstp, src_ap[:, :nj, :])
                        else:
                            for j in range(nj):
                                mj = min(128, nrows - j * 128)
                                nc.scalar.dma_start(
                                    tbl[rb + j * 128:rb + j * 128 + mj, :],
                                    src_ap[:mj, j, :])
                        return
                    for j in range(nj):
                        mj = min(128, nrows - j * 128)
                        rj = r0 + j * 128
                        if rj + mj <= LO:
                            nc.scalar.dma_start(table_lo[rj:rj + mj, :],
                                                src_ap[:mj, j, :])
                        elif rj >= LO:
                            nc.scalar.dma_start(table_hi[rj - LO:rj - LO + mj, :],
                                                src_ap[:mj, j, :])
                        else:
                            cut = LO - rj
                            nc.scalar.dma_start(table_lo[rj:LO, :],
                                                src_ap[:cut, j, :])
                            nc.scalar.dma_start(table_hi[0:mj - cut, :],
                                                src_ap[cut:mj, j, :])

                def finalize_window(wdx):
                    dn = sp.tile([128, 4], f32, tag="dn", name=f"dn{wdx}")
                    nc.scalar.activation(dn[:], slab[:, wdx, 256:260], AF.Identity, bias=e30[:, :])
                    rd = sp.tile([128, 4], f32, tag="rd", name=f"rd{wdx}")
                    nc.vector.reciprocal(rd[:], dn[:])
                    tt = sp.tile([128, 256], bf16, tag="tt", name=f"tt{wdx}")
                    nc.vector.tensor_tensor(
                        out=tt[:].rearrange("p (h d) -> p h d", h=4),
                        in0=slab[:, wdx, :256].rearrange("p (h d) -> p h d", h=4),
                        in1=rd[:].broadcast_to([128, 4, 64]),
                        op=AL.mult)
                    ttsb = sp.tile([128, 2, 128], bf16, tag="ttsb", name=f"ttsb{wdx}")
                    yps = ypp.tile([128, D], f32, tag="yps", name=f"yps{wdx}")
                    for k in range(2):
                        ttp = tpp.tile([128, 128], bf16, tag="ttp",
                                       name=f"ttp{wdx}_{k}")
                        nc.tensor.transpose(ttp[:], tt[:, k * 128:(k + 1) * 128],
                                            ident[:])
                        nc.scalar.copy(ttsb[:, k, :], ttp[:])
                        nc.tensor.matmul(out=yps[:], lhsT=ttsb[:, k, :],
                                         rhs=wstk_sb[:, k, :],
                                         start=(k == 0), stop=(k == 1))
                    nc.vector.scalar_tensor_tensor(
                        out=y_sb[:, wdx, :], in0=yps[:], scalar=0.25, in1=bias_sb[:],
                        op0=AL.mult, op1=AL.add)
                    sq = sp.tile([128, D], f32, tag="sq", name=f"sq{wdx}")
                    nc.scalar.square(sq[:], y_sb[:, wdx, :])
                    msk = masks_sb[:, wdx:wdx + 1]
                    nc.tensor.matmul(out=bn_s[:], lhsT=msk, rhs=y_sb[:, wdx, :],
                                     start=(wdx == 0), stop=(wdx == NW - 1))
                    nc.tensor.matmul(out=bn_q[:], lhsT=msk, rhs=sq[:],
                                     start=(wdx == 0), stop=(wdx == NW - 1))

                wpt_by_sec = {}

                def emit_call(ci):
                    kind = sched["call_kind"][ci]
                    tbl = table_lo[:, :] if kind == 0 else table_hi[:, :]
                    q = ci % 4
                    live = [(b, binfo[ci * BPC + b]) for b in range(BPC)
                            if not binfo[ci * BPC + b]["dead"]]
                    if not live:
                        return
                    nb = live[-1][0] + 1  # dead batches are a strict suffix
                    nidx = nb * 128
                    gt = gp.tile([128, BPC, RE], bf16, tag="g")
                    nc.gpsimd.dma_gather(
                        out_ap=gt[:, :nb, :], in_ap=tbl,
                        idxs_ap=gidx_sb[:, ci * (CALL // 16):
                                        ci * (CALL // 16) + nidx // 16],
                        num_idxs=nidx, num_idxs_reg=nidx, elem_size=RE,
                        queue_num=q, single_packet=False)
                    indall = ip.tile([128, BPC * 256], fp8, tag="ind")
                    nc.sync.dma_start(indall[:], INDB[:, ci * BPC * 256:(ci + 1) * BPC * 256])
                    ind_t = indall[:, :BPC * 128]
                    indt_t = indall[:, BPC * 128:]

                    adst_pt = app.tile([128, BPC, 4], f32, tag="adst")
                    for b, info in live:
                        nc.tensor.matmul(
                            out=adst_pt[:, b, :],
                            lhsT=indt_t[:, b * 128:(b + 1) * 128],
                            rhs=a_dst[:, info["w"], :],
                            start=True, stop=True)
                    e0 = sp.tile([128, BPC, 4], f32, tag="e0")
                    nc.vector.tensor_tensor(
                        out=e0[:, :nb], in0=gt[:].bitcast(f32)[:, :nb, 32:36],
                        in1=adst_pt[:, :nb], op=AL.add)
                    e1 = sp.tile([128, BPC, 4], f32, tag="e1")
                    nc.vector.scalar_tensor_tensor(
                        out=e1[:, :nb], in0=e0[:, :nb], scalar=NEG, in1=e0[:, :nb],
                        op0=AL.mult, op1=AL.max)
                    msg = mp.tile([128, BPC, 260], bf16, tag="msg")
                    nc.scalar.activation(msg[:, :nb, 256:260], e1[:, :nb], AF.Exp)
                    nc.vector.tensor_tensor(
                        out=msg[:, :nb, :256].rearrange("p c (h d) -> p c h d", h=4),
                        in0=gt[:, :nb, 0:64].unsqueeze(2).broadcast_to(
                            [128, nb, 4, 64]),
                        in1=msg[:, :nb, 256:260].unsqueeze(3).broadcast_to(
                            [128, nb, 4, 64]),
                        op=AL.mult)
                    for b, info in live:
                        s = info["sec"]
                        if info["start"]:
                            wpt_by_sec[s] = wp.tile([128, 260], f32, tag="wpt", name=f"wpt{s}")
                        nc.tensor.matmul(
                            out=wpt_by_sec[s][:],
                            lhsT=ind_t[:, b * 128:(b + 1) * 128],
                            rhs=msg[:, b, :],
                            start=info["start"], stop=info["stop"])
                        if info["stop"]:
                            wdx = info["w"]
                            if info["kind"] == 0:
                                nc.scalar.copy(slab[:, wdx, :], wpt_by_sec[s][:])
                            else:
                                nc.vector.tensor_tensor(
                                    out=slab[:, wdx, :], in0=slab[:, wdx, :],
                                    in1=wpt_by_sec[s][:], op=AL.add)
                            del wpt_by_sec[s]
                            if info["kind"] == 1:
                                finalize_window(wdx)

                n_iters = (N + 1023) // 1024
                lo_iters = (LO + 1023) // 1024  # chunks covering the lo half
                lo_calls = [ci for ci in range(NCALLS) if sched["call_kind"][ci] == 0]
                hi_calls = [ci for ci in range(NCALLS) if sched["call_kind"][ci] == 1]

                with tc.tile_pool(name="pt_ps", bufs=2, space="PSUM") as tps, \
                     tc.tile_pool(name="pt_ps2", bufs=1, space="PSUM") as tps2, \
                     tc.tile_pool(name="pt_ps3", bufs=1, space="PSUM") as tps3:

                    def phase_a():
                        # a_dst for own slab (from xTs, padded to NW*128)
                        for t in range((NW * 128 + 511) // 512):
                            c0 = t * 512
                            nt = min(512, NW * 128 - c0)
                            xta = tp.tile([D, 512], bf16, tag="xta")
                            nc.sync.dma_start(xta[:, :nt], xTs[:, c0:c0 + nt])
                            ma = tps.tile([128, 512], f32, tag="m1")
                            nc.tensor.matmul(out=ma[:D, :nt], lhsT=w1t_sb[:],
                                             rhs=xta[:, :nt], start=True, stop=True)
                            x1a = tp.tile([D, 512], bf16, tag="x1a")
                            prelu_apply(x1a[:, :nt], ma[:D, :nt], D, nt, "m1ca")
                            adp = tps3.tile([128, 4, 8], f32, tag="as")
                            j = 0
                            while j * 128 < nt:
                                nc.tensor.matmul(out=adp[:, j, 0:4],
                                                 lhsT=x1a[:, j * 128:(j + 1) * 128],
                                                 rhs=cd_sb[:], start=True, stop=True)
                                j += 1
                            w0 = c0 // 128
                            nc.vector.tensor_copy(a_dst[:, w0:w0 + j, :], adp[:, :j, 0:4])

                    def chunk_body(t):
                        c0 = t * 1024
                        nt = min(1024, N - c0)  # 1024 or 848 on last
                        na = min(512, nt)
                        nb_ = nt - na
                        xt = tp.tile([128, 512], bf16, tag="xt")
                        if nb_ == 512:
                            nc.sync.dma_start(xt[:64, :], xT[:, c0:c0 + 512])
                            nc.sync.dma_start(xt[64:, :], xT[:, c0 + 512:c0 + 1024])
                        else:
                            nc.vector.memset(xt[64:, :], 0.0)
                            nc.sync.dma_start(xt[:64, :na], xT[:, c0:c0 + na])
                            if nb_ > 0:
                                nc.sync.dma_start(xt[64:, :nb_],
                                                  xT[:, c0 + 512:c0 + 512 + nb_])
                        m1 = tps.tile([128, 512], f32, tag="m1")
                        nc.tensor.matmul(out=m1[:], lhsT=w1b_sb[:], rhs=xt[:],
                                         start=True, stop=True)
                        x1 = tp.tile([128, 512], bf16, tag="x1")
                        prelu_apply(x1[:], m1[:], 128, 512, "m1c")
                        tpall = tps2.tile([128, 4, 128], bf16, tag="tp")
                        asall = tps3.tile([128, 4, 8], f32, tag="as")
                        nja = (na + 127) // 128
                        for j in range(4):
                            if j * 128 >= na and j * 128 >= nb_:
                                break
                            nc.tensor.transpose(tpall[:, j, :],
                                                x1[:, j * 128:(j + 1) * 128], ident[:])
                            nc.tensor.matmul(out=asall[:, j, :],
                                             lhsT=x1[:, j * 128:(j + 1) * 128],
                                             rhs=cs2_sb[:], start=True, stop=True)
                        rowb = tp.tile([128, 2, 4, RE], bf16, tag="rowb")
                        nc.vector.tensor_copy(rowb[:, 0, :nja, 0:64], tpall[:, :nja, 0:64])
                        nc.vector.tensor_copy(
                            rowb[:].bitcast(f32)[:, 0, :nja, 32:36],
                            asall[:, :nja, 0:4])
                        if nb_ > 0:
                            njb = (nb_ + 127) // 128
                            nc.vector.tensor_copy(rowb[:, 1, :njb, 0:64],
                                                  tpall[:, :njb, 64:128])
                            nc.vector.tensor_copy(
                                rowb[:].bitcast(f32)[:, 1, :njb, 32:36],
                                asall[:, :njb, 4:8])
                        if nt == 1024 and (c0 + 1024 <= LO or c0 >= LO):
                            store_span(c0, 1024,
                                       rowb[:].rearrange("p k j e -> p (k j) e"))
                        else:
                            store_span(c0, na, rowb[:, 0, :, :])
                            if nb_ > 0:
                                store_span(c0 + 512, nb_, rowb[:, 1, :, :])

                    # pad rows first
                    padrow = tp.tile([1, RE], bf16, tag="pad")
                    nc.vector.memset(padrow[:], 0.0)
                    nc.vector.memset(padrow[:].bitcast(f32)[:, 32:36], -1e4)
                    nc.sync.dma_start(table_lo[LO:LO + 1, :], padrow[:])
                    nc.sync.dma_start(table_hi[LO:LO + 1, :], padrow[:])

                    phase_a()
                    for t in range(lo_iters):
                        chunk_body(t)
                    # lo gathers start here (all table_lo writes are emitted);
                    # hi chunks stream concurrently on the compute engines.
                    hi_chunks = list(range(lo_iters, n_iters))
                    for i, ci in enumerate(lo_calls):
                        emit_call(ci)
                        if i < len(hi_chunks):
                            chunk_body(hi_chunks[i])
                    for t in hi_chunks[len(lo_calls):]:
                        chunk_body(t)

                with tc.tile_pool(name="pe_tp", bufs=1, space="PSUM") as tpp, \
                     tc.tile_pool(name="pe_yp", bufs=1, space="PSUM") as ypp, \
                     tc.tile_pool(name="pf_ps", bufs=1, space="PSUM") as fps:
                    bn_s = fps.tile([1, D], f32, tag="bns")
                    bn_q = fps.tile([1, D], f32, tag="bnq")

                    for ci in hi_calls:
                        emit_call(ci)

                    # ---------------- phase B: BN + relu + store ---------------
                    fp_ = sp
                    st = fp_.tile([1, 128], f32, tag="st")
                    nc.vector.tensor_copy(st[:, :64], bn_s[:])
                    nc.vector.tensor_copy(st[:, 64:], bn_q[:])
                    nc.sync.dma_start(cc_in[:], st[:])
                    nc.gpsimd.collective_compute(
                        "AllReduce", AL.add, replica_groups=[list(range(NC))],
                        ins=[cc_in[:].opt()], outs=[cc_out[:].opt()])
                    st2 = fp_.tile([1, 128], f32, tag="st2")
                    nc.sync.dma_start(st2[:], cc_out[:])
                    mean = fp_.tile([1, D], f32, tag="mean")
                    nc.scalar.mul(mean[:], st2[:, :64], 1.0 / N)
                    ex2 = fp_.tile([1, D], f32, tag="ex2")
                    nc.scalar.mul(ex2[:], st2[:, 64:], 1.0 / N)
                    msq = fp_.tile([1, D], f32, tag="msq")
                    nc.scalar.square(msq[:], mean[:])
                    var = fp_.tile([1, D], f32, tag="var")
                    nc.vector.tensor_tensor(out=var[:], in0=ex2[:], in1=msq[:],
                                            op=AL.subtract)
                    sd = fp_.tile([1, D], f32, tag="sd")
                    nc.scalar.activation(sd[:], var[:], AF.Sqrt, bias=ebn[:, :])
                    rs = fp_.tile([1, D], f32, tag="rs")
                    nc.vector.reciprocal(rs[:], sd[:])
                    scsh = fp_.tile([1, 128], f32, tag="scsh")
                    nc.vector.tensor_tensor(out=scsh[:, :64], in0=gb_sb[:, :64], in1=rs[:],
                                            op=AL.mult)
                    mssc = fp_.tile([1, D], f32, tag="mssc")
                    nc.vector.tensor_tensor(out=mssc[:], in0=mean[:], in1=scsh[:, :64],
                                            op=AL.mult)
                    nc.vector.tensor_tensor(out=scsh[:, 64:], in0=gb_sb[:, 64:], in1=mssc[:],
                                            op=AL.subtract)
                    bc = sp.tile([128, 128], f32, tag="bc")
                    nc.gpsimd.partition_broadcast(bc[:], scsh[:])
                    z = zp.tile([128, NW, D], f32, tag="za")
                    nc.vector.tensor_tensor(
                        out=z[:],
                        in0=y_sb[:], in1=bc[:, :64].unsqueeze(1).broadcast_to(
                            [128, NW, 64]),
                        op=AL.mult)
                    z2 = zp.tile([128, NW, D], f32, tag="zb")
                    nc.vector.tensor_tensor(
                        out=z2[:], in0=z[:], in1=bc[:, 64:].unsqueeze(1).broadcast_to(
                            [128, NW, 64]),
                        op=AL.add)
                    zo = zp.tile([128, NW, D], f32, tag="za", name="zo")
                    nc.scalar.activation(zo[:], z2[:], AF.Relu)
                    nc.sync.dma_start(
                        out_slab[:, :].rearrange("(w p) d -> p w d", p=128),
                        zo[:, :, :])

    nc.compile()
    return nc


def kernel(x, edge_index, W_lin, b_lin, prelu_w, W_gat, att_src, att_dst,
           gat_bias, bn_gamma, bn_beta):
    global LAST_EXEC_NS, LAST_TRACE
    from concourse import bass_utils

    x = np.asarray(x, np.float32)
    edge_index = np.asarray(edge_index)
    W_lin = np.asarray(W_lin, np.float32)
    b_lin = np.asarray(b_lin, np.float32)
    prelu_w = np.asarray(prelu_w, np.float32)
    W_gat = np.asarray(W_gat, np.float32)
    att_src = np.asarray(att_src, np.float32)
    att_dst = np.asarray(att_dst, np.float32)
    gat_bias = np.asarray(gat_bias, np.float32)
    bn_gamma = np.asarray(bn_gamma, np.float32)
    bn_beta = np.asarray(bn_beta, np.float32)

    # b_lin is zero in the reference setup; if nonzero, do the pre-linear
    # exactly on host and feed the device an identity pre-stage.
    if np.any(b_lin != 0):
        x1_host = x @ W_lin.T + b_lin
        x1_host = np.where(x1_host >= 0, x1_host, prelu_w * x1_host)
        xT_eff = np.ascontiguousarray(x1_host.T)
        W1_eff = np.eye(64, dtype=np.float32)
        prelu_eff = np.ones((64,), np.float32)
    else:
        xT_eff = np.ascontiguousarray(x.T)
        W1_eff = W_lin
        prelu_eff = prelu_w

    prelu_uniform = bool(np.all(prelu_eff == prelu_eff[0]))
    prelu_value = float(prelu_eff[0]) if prelu_uniform else 0.0

    key = (hashlib.sha1(np.ascontiguousarray(edge_index).tobytes()).hexdigest(),
           prelu_uniform, prelu_value)
    if key not in _CACHE:
        sched, blobs = _schedule_and_blobs(edge_index)
        nc = _build_program(sched, prelu_uniform, prelu_value)
        _CACHE[key] = (sched, blobs, nc)
    sched, blobs, nc = _CACHE[key]

    C_src = np.zeros((64, 4), np.float32)
    C_dst = np.zeros((64, 4), np.float32)
    for h in range(H):
        Wh = W_gat[h * 64:(h + 1) * 64, :]  # [64, 64] maps x1 -> head h
        C_src[:, h] = Wh.T @ att_src[h]
        C_dst[:, h] = Wh.T @ att_dst[h]

    bf = ml_dtypes.bfloat16
    W1T_np = np.ascontiguousarray(W1_eff.T).astype(bf)  # [din, dout]
    W1B_np = np.zeros((128, 128), np.float32)
    W1B_np[:64, :64] = W1_eff.T
    W1B_np[64:, 64:] = W1_eff.T
    CS2_np = np.zeros((128, 8), np.float32)
    CS2_np[:64, 0:4] = C_src
    CS2_np[64:, 4:8] = C_src
    # WSTK[k*128+p, d'] laid out [128, 2*64]: row p, block k: W_h.T stacked
    # rows hd = h*64+dk -> Wstk[h*64+dk, d'] = W_gat[h*64+d', dk]
    WSTK_np = np.zeros((256, 64), np.float32)
    for h in range(H):
        WSTK_np[h * 64:(h + 1) * 64, :] = W_gat[h * 64:(h + 1) * 64, :].T
    WSTK_2 = np.concatenate([WSTK_np[:128], WSTK_np[128:]], axis=1)  # [128, 128]

    prelu2_np = np.concatenate([prelu_eff, prelu_eff]).reshape(128, 1)
    xT_bf = xT_eff.astype(bf)

    perm_core = sched["perm_core"]
    perm_slot = sched["perm_slot"]
    in_maps = []
    for c in range(NC):
        xs = np.zeros((64, NW * 128), np.float32)
        own = np.nonzero(perm_core == c)[0]
        xs[:, perm_slot[own]] = xT_eff[:, own]
        msks = np.zeros((NW * 128,), np.float32)
        msks[perm_slot[own]] = 1.0
        msks = np.ascontiguousarray(msks.reshape(NW, 128).T)
        in_maps.append(dict(
            xT=xT_bf,
            xTs=xs.astype(bf),
            W1B=W1B_np.astype(bf),
            W1T=W1T_np,
            CS2=CS2_np.astype(bf),
            CD=C_dst.astype(bf),
            WSTK=WSTK_2.astype(bf),
            prelu2=prelu2_np,
            prelu1=prelu_eff.reshape(64, 1),
            GIDX=blobs[c]["GIDX"], INDB=blobs[c]["INDB"],
            bias128=np.tile(gat_bias[None, :], (128, 1)),
            MASKS=msks,
            gb_row=np.concatenate([bn_gamma, bn_beta])[None, :],
        ))

    trace = os.environ.get("GAT_TRACE", "0") == "1"
    if trace:
        _install_ntff_shim()
    res = bass_utils.run_bass_kernel_spmd(nc, in_maps, core_ids=list(range(NC)),
                                          trace=trace)
    LAST_EXEC_NS = res.exec_time_ns
    LAST_TRACE = res.instructions_and_trace
    out = np.empty((N, D), np.float32)
    for c in range(NC):
        own = np.nonzero(perm_core == c)[0]
        out[own] = res.results[c]["out_slab"][perm_slot[own]]
    return out



# revision 24
# speedup vs baseline: 1.1173x; 1.1173x over previous
"""GAT-mod forward on 8 trn2 NeuronCores (Bass/Tile).

Strategy (dst-sharded, x1-space aggregation):
- Nodes are partitioned across 8 cores by destination id (6250 each).
- Key identity: h = W_gat @ x1 is linear, so the GAT aggregation
  out[n,h,:] = sum_e alpha_e * h[src_e,h,:] = W_h @ (sum_e alpha_e x1[src_e]).
  We aggregate in x1-space (64 wide per head-weight, 4 heads share the same
  x1) and apply W_gat per 128-node window AFTER normalization.
- Each core builds the full node table T[n] = [x1 bf16(64) | a_src f32(4) | pad]
  (256B rows, the dma_gather minimum) in its local HBM, split at row 25000 so
  gather indices fit int16, with a PAD row per half (x1=0, a_src=-1e4).
- Edges (incl. self-loops) are grouped by 128-node destination windows, split
  into lo/hi source halves, packed into 128-slot batches; 16 batches per
  gather call (2048 idxs). Gathers use prepare_only+trigger_dma so gpsimd only
  pays descriptor generation; transfers run async on the DMA queues.
  Per batch: e = lrelu(a_src + IndT@a_dst); p = exp(e) (no-max softmax, e is
  bounded); msg = [x1*p per head | p] (260 wide); PSUM[node,260] += Ind^T@msg.
- Per window finalize: z/denom -> 2 PE transposes -> 2 matmuls with stacked
  W_gat^T -> y = 0.25*sum_h + bias; BN stats via per-window PE matmuls.
- BN batch stats via AllReduce across the 8 cores; bulk BN apply + store.
"""

import os
import sys
import hashlib

import numpy as np
import ml_dtypes

N = 50000
E = 800000
D = 64
H = 4
NEG = 0.2
BN_EPS = 1e-5
NC = 8
SLAB = N // NC          # 6250
W = 128                 # window node slots
NW = 53                 # windows per core (edge-balanced, ~118 nodes each)
LO = 25000
RE = 128                # table row elems (bf16): 64 x1 + 8 (4 f32 a_src) + pad
TROWS = 2 * LO + 2      # 50002 (two pad rows)
PAD_LO = LO             # pad row index within lo half
PAD_HI = LO             # within hi half (row 25001+25000 = 50001)
BPC = 8                 # batches per gather call
CALL = BPC * 128        # 1024 idxs per gather (HW ucode caps at 1024)

_CACHE = {}
LAST_EXEC_NS = None
LAST_TRACE = None


def _install_ntff_shim():
    import contextlib
    import ctypes
    import types

    if "antenv.axon_hooks" in sys.modules:
        return
    so_path = "/opt/axon/libaxon_pjrt.so"

    def _hook_factory(so_path):
        try:
            lib = ctypes.CDLL(so_path)
        except OSError:
            return None
        if not hasattr(lib, "axon_start_nrt_profile"):
            return None
        lib.axon_start_nrt_profile.argtypes = [ctypes.POINTER(ctypes.c_int64), ctypes.c_size_t]
        lib.axon_start_nrt_profile.restype = ctypes.c_int64
        lib.axon_stop_nrt_profile.argtypes = [ctypes.c_char_p]
        lib.axon_stop_nrt_profile.restype = ctypes.c_int64

        @contextlib.contextmanager
        def _hook(output_dir, device_ids):
            import jax

            jax.devices()
            if device_ids:
                ids = (ctypes.c_int64 * len(device_ids))(*device_ids)
                rc = lib.axon_start_nrt_profile(ids, len(device_ids))
            else:
                rc = lib.axon_start_nrt_profile(None, 0)
            if rc != 0:
                raise RuntimeError(f"axon_start_nrt_profile rc={rc}")
            try:
                yield
            finally:
                lib.axon_stop_nrt_profile(str(output_dir).encode())

        return _hook

    mod = types.ModuleType("antenv.axon_hooks")
    _h = [None]
    mod.set_axon_ntff_profile_hook = lambda h: _h.__setitem__(0, h)
    mod.get_axon_ntff_profile_hook = lambda: _h[0]
    sys.modules["antenv.axon_hooks"] = mod
    try:
        import antenv

        antenv.axon_hooks = mod
    except ImportError:
        pass
    mod.set_axon_ntff_profile_hook(_hook_factory(so_path))


# ----------------------------------------------------------------- host prep
def _node_permutation(edge_index):
    """Greedy edge-balanced packing of nodes into (window, core) buckets:
    minimizes the max per-(window, src-half) edge count so every section
    fits its batch budget on every core."""
    src = np.concatenate([edge_index[0].astype(np.int64), np.arange(N, dtype=np.int64)])
    dst = np.concatenate([edge_index[1].astype(np.int64), np.arange(N, dtype=np.int64)])
    d_lo = np.bincount(dst[src < LO], minlength=N).astype(np.int64)
    d_hi = np.bincount(dst[src >= LO], minlength=N).astype(np.int64)
    order = np.argsort(-(d_lo + d_hi), kind="stable")
    nbk = NW * NC
    fill_n = np.zeros(nbk, np.int64)
    fill_l = np.zeros(nbk, np.float64)
    fill_h = np.zeros(nbk, np.float64)
    bucket_core = np.tile(np.arange(NC), NW)
    perm_core = np.empty(N, np.int64)
    perm_slot = np.empty(N, np.int64)
    for n in order:
        score = np.maximum(fill_l + d_lo[n], fill_h + d_hi[n])
        score = np.where(fill_n >= 128, 1e18, score)
        b = int(np.argmin(score))
        perm_core[n] = bucket_core[b]
        perm_slot[n] = (b // NC) * 128 + fill_n[b]
        fill_n[b] += 1
        fill_l[b] += d_lo[n]
        fill_h[b] += d_hi[n]
    return perm_core, perm_slot


def _schedule_and_blobs(edge_index):
    src = np.concatenate([edge_index[0].astype(np.int64), np.arange(N, dtype=np.int64)])
    dst = np.concatenate([edge_index[1].astype(np.int64), np.arange(N, dtype=np.int64)])
    perm_core, perm_slot = _node_permutation(edge_index)

    cores = []
    for c in range(NC):
        sel = perm_core[dst] == c
        s_src = src[sel]
        s_dst = perm_slot[dst[sel]]
        islo = s_src < LO
        win = s_dst >> 7
        secid = win * 2 + (1 - islo.astype(np.int64))  # even = lo, odd = hi
        order = np.argsort(secid, kind="stable")
        cores.append((s_src[order], s_dst[order], secid[order]))

    # per-(core, section) counts; shared schedule = max over cores
    NSEC = NW * 2
    cnts = np.zeros((NC, NSEC), np.int64)
    for c in range(NC):
        binc = np.bincount(cores[c][2], minlength=NSEC)
        cnts[c] = binc
    nb_sec = (np.max(cnts, axis=0) + 127) // 128  # batches per section
    nb_sec = np.maximum(nb_sec, 1)

    # batch list: lo run (even sections, w ascending), then hi run
    batches = []  # (w, kind, sec, dead)
    for kind in (0, 1):  # 0=lo, 1=hi
        run_start = len(batches)
        for wdx in range(NW):
            s = wdx * 2 + kind
            for _ in range(int(nb_sec[s])):
                batches.append([wdx, kind, s, False])
        while (len(batches) - run_start) % BPC != 0:
            batches.append([0, kind, -1, True])
    NB = len(batches)
    NCALLS = NB // BPC

    # within-call permutation: move each section's (globally) final batch and
    # dead batches to the back of their call so their pad slots form a
    # trailing run that the gather ucode's negative-index trim can skip.
    sec_last_global = {}
    for bi, (wdx, kind, s, dead) in enumerate(batches):
        if not dead:
            sec_last_global[s] = bi
    order = []
    for ci in range(NCALLS):
        idxs = list(range(ci * BPC, (ci + 1) * BPC))
        front = [b for b in idxs
                 if not batches[b][3] and sec_last_global[batches[b][2]] != b]
        back = [b for b in idxs
                if batches[b][3] or sec_last_global[batches[b][2]] == b]
        order += front + back
    batches = [batches[b] for b in order]
    call_kind = [batches[ci * BPC][1] for ci in range(NCALLS)]

    # mark section start/stop per batch (execution order = list order)
    sec_first = {}
    sec_last = {}
    for bi, (wdx, kind, s, dead) in enumerate(batches):
        if dead:
            continue
        if s not in sec_first:
            sec_first[s] = bi
        sec_last[s] = bi
    binfo = []
    for bi, (wdx, kind, s, dead) in enumerate(batches):
        binfo.append(dict(w=wdx, kind=kind, sec=s, dead=dead,
                          start=(not dead and sec_first[s] == bi),
                          stop=(not dead and sec_last[s] == bi)))

    sched = dict(NB=NB, NCALLS=NCALLS, call_kind=call_kind, binfo=binfo)

    # per-core blobs
    blobs = []
    for c in range(NC):
        s_src, s_dst, s_sec = cores[c]
        gidx = np.full((NB * 128,), PAD_LO, np.int64)
        filled = np.zeros((NB * 128,), bool)
        ind = np.zeros((NB, 128, 128), np.float32)
        indt = np.zeros((NB, 128, 128), np.float32)
        # per-section edge ranges; edges fill each section's batches in
        # execution order
        sec_edge_start = np.zeros(NSEC + 1, np.int64)
        np.cumsum(np.bincount(s_sec, minlength=NSEC), out=sec_edge_start[1:])
        sec_ptr = sec_edge_start[:-1].copy()
        for bi, info in enumerate(binfo):
            if info["dead"]:
                continue
            s = info["sec"]
            e0 = int(sec_ptr[s])
            e1 = min(e0 + 128, int(sec_edge_start[s + 1]))
            n = e1 - e0
            sec_ptr[s] = e1
            if n <= 0:
                continue
            srcs = s_src[e0:e1]
            idxv = srcs if info["kind"] == 0 else srcs - LO
            gidx[bi * 128:bi * 128 + n] = idxv
            filled[bi * 128:bi * 128 + n] = True
            node_in_w = (s_dst[e0:e1] - info["w"] * 128).astype(np.int64)
            sloc = np.arange(n)
            ind[bi, sloc, node_in_w] = 1.0
            indt[bi, node_in_w, sloc] = 1.0
        # trailing pads of each call become -1 (ucode trims them); keep the
        # first 8 calls on the PAD row since their gt buffers start
        # uninitialized (later calls see finite stale data, safe).
        if os.environ.get("GAT_NOTRIM", "0") != "1":
            for ci in range(12, NCALLS):
                blk = filled[ci * CALL:(ci + 1) * CALL]
                nz = np.nonzero(blk)[0]
                last = nz[-1] if len(nz) else -1
                gidx[ci * CALL + last + 1:(ci + 1) * CALL] = -1
        # wrap gather indices: call ci covers positions [ci*CALL, +CALL)
        g16 = gidx.astype(np.int16).reshape(NCALLS, CALL // 16, 16)
        gw = np.transpose(g16, (0, 2, 1)).reshape(NCALLS, 16, CALL // 16)
        gw = np.tile(gw, (1, 8, 1))  # [NCALLS, 128, CALL//16]
        GIDX = np.ascontiguousarray(
            np.transpose(gw, (1, 0, 2)).reshape(128, NCALLS * (CALL // 16)))
        both = np.concatenate([ind.reshape(NCALLS, BPC, 128, 128),
                               indt.reshape(NCALLS, BPC, 128, 128)], axis=1)
        INDB = np.ascontiguousarray(
            np.transpose(both, (2, 0, 1, 3)).reshape(128, NB * 256)).astype(ml_dtypes.float8_e4m3)
        blobs.append(dict(GIDX=GIDX, INDB=INDB))
    sched["perm_core"] = perm_core
    sched["perm_slot"] = perm_slot
    return sched, blobs


def _build_program(sched, prelu_uniform=True, prelu_value=0.25):
    from concourse import bacc, masks, mybir
    from concourse.tile import TileContext

    AL = mybir.AluOpType
    AF = mybir.ActivationFunctionType
    f32 = mybir.dt.float32
    bf16 = mybir.dt.bfloat16
    fp8 = mybir.dt.float8e4
    i16 = mybir.dt.int16

    NB = sched["NB"]
    NCALLS = sched["NCALLS"]
    binfo = sched["binfo"]

    nc = bacc.Bacc("TRN2", target_bir_lowering=False, debug=False,
                   num_devices=NC, num_swdge_queues=4)

    xT = nc.dram_tensor("xT", (D, N), bf16, kind="ExternalInput")
    xTs = nc.dram_tensor("xTs", (D, NW * 128), bf16, kind="ExternalInput")
    W1B = nc.dram_tensor("W1B", (128, 128), bf16, kind="ExternalInput")
    W1T = nc.dram_tensor("W1T", (D, D), bf16, kind="ExternalInput")
    CS2 = nc.dram_tensor("CS2", (128, 8), bf16, kind="ExternalInput")
    CD = nc.dram_tensor("CD", (D, 4), bf16, kind="ExternalInput")
    WSTK = nc.dram_tensor("WSTK", (128, 2 * D), bf16, kind="ExternalInput")
    prelu2 = nc.dram_tensor("prelu2", (128, 1), f32, kind="ExternalInput")
    prelu1 = nc.dram_tensor("prelu1", (D, 1), f32, kind="ExternalInput")
    GIDX = nc.dram_tensor("GIDX", (128, NCALLS * (CALL // 16)), i16, kind="ExternalInput")
    INDB = nc.dram_tensor("INDB", (128, NB * 256), fp8, kind="ExternalInput")
    bias128 = nc.dram_tensor("bias128", (128, D), f32, kind="ExternalInput")
    MASKS = nc.dram_tensor("MASKS", (128, NW), f32, kind="ExternalInput")
    gb_row = nc.dram_tensor("gb_row", (1, 128), f32, kind="ExternalInput")  # [gamma|beta]
    out_slab = nc.dram_tensor("out_slab", (NW * 128, D), f32, kind="ExternalOutput")

    with TileContext(nc) as tc:
        with tc.tile_pool(name="dram", bufs=1, space="DRAM") as dpool, \
             tc.tile_pool(name="dram_lo", bufs=1, space="DRAM") as dpool_lo, \
             tc.tile_pool(name="dram_hi", bufs=1, space="DRAM") as dpool_hi, \
             tc.tile_pool(name="persist", bufs=1) as pp:
            table_lo = dpool_lo.tile([LO + 1, RE], bf16)
            table_hi = dpool_hi.tile([LO + 1, RE], bf16)
            cc_in = dpool.tile([1, 128], f32)
            cc_out = dpool.tile([1, 128], f32)

            w1b_sb = pp.tile([128, 128], bf16)
            nc.sync.dma_start(w1b_sb[:], W1B[:, :])
            w1t_sb = pp.tile([D, D], bf16)
            nc.sync.dma_start(w1t_sb[:], W1T[:, :])
            cs2_sb = pp.tile([128, 8], bf16)
            nc.sync.dma_start(cs2_sb[:], CS2[:, :])
            cd_sb = pp.tile([D, 4], bf16)
            nc.sync.dma_start(cd_sb[:], CD[:, :])
            wstk_sb = pp.tile([128, 2, D], bf16)
            nc.sync.dma_start(wstk_sb[:].rearrange("p a b -> p (a b)"), WSTK[:, :])
            prelu2_sb = pp.tile([128, 1], f32)
            nc.sync.dma_start(prelu2_sb[:], prelu2[:, :])
            prelu1_sb = pp.tile([D, 1], f32)
            nc.sync.dma_start(prelu1_sb[:], prelu1[:, :])
            bias_sb = pp.tile([128, D], f32)
            nc.sync.dma_start(bias_sb[:], bias128[:, :])
            masks_sb = pp.tile([128, NW], f32)
            nc.sync.dma_start(masks_sb[:], MASKS[:, :])
            gb_sb = pp.tile([1, 128], f32)
            nc.sync.dma_start(gb_sb[:], gb_row[:, :])
            gidx_sb = pp.tile([128, NCALLS * (CALL // 16)], i16)
            nc.sync.dma_start(gidx_sb[:], GIDX[:, :])
            ident = pp.tile([128, 128], bf16)
            masks.make_identity(nc, ident[:])
            e30 = pp.tile([128, 1], f32)
            nc.vector.memset(e30[:], 1e-30)
            ebn = pp.tile([1, 1], f32)
            nc.vector.memset(ebn[:], BN_EPS)

            a_dst = pp.tile([128, NW, 4], bf16)
            slab = pp.tile([128, NW, 260], f32)
            y_sb = pp.tile([128, NW, D], f32)

            # -------- phase A + lo table, then lo-gathers ‖ hi table, then hi ----
            with tc.tile_pool(name="pt_sb", bufs=3) as tp, \
                 tc.tile_pool(name="pe_g", bufs=12) as gp, \
                 tc.tile_pool(name="pe_i", bufs=4) as ip, \
                 tc.tile_pool(name="pe_s", bufs=3) as sp, \
                 tc.tile_pool(name="pe_m", bufs=5) as mp, \
                 tc.tile_pool(name="pe_z", bufs=1) as zp, \
                 tc.tile_pool(name="pe_wp", bufs=2, space="PSUM") as wp, \
                 tc.tile_pool(name="pe_ap", bufs=1, space="PSUM") as app:

                def prelu_apply(x1_out, m1_in, np_, nw_, tag):
                    # m1_in is PSUM f32; copy to bf16 SBUF on Scalar, then one
                    # DVE op computes max(w*m, m) on the 16-bit copy.
                    m1c = tp.tile([128, 512], bf16, tag=tag)
                    nc.scalar.copy(m1c[:np_, :nw_], m1_in)
                    if prelu_uniform:
                        nc.vector.scalar_tensor_tensor(
                            out=x1_out, in0=m1c[:np_, :nw_], scalar=prelu_value,
                            in1=m1c[:np_, :nw_], op0=AL.mult, op1=AL.max)
                    else:
                        pw = prelu2_sb if np_ == 128 else prelu1_sb
                        tmp = tp.tile([128, 512], bf16, tag=tag + "w")
                        nc.vector.tensor_tensor(
                            out=tmp[:np_, :nw_], in0=m1c[:np_, :nw_],
                            in1=pw[:, :].broadcast_to([np_, nw_]),
                            op=AL.mult)
                        nc.vector.tensor_tensor(out=x1_out, in0=tmp[:np_, :nw_],
                                                in1=m1c[:np_, :nw_], op=AL.max)

                def store_span(r0, nrows, src_ap):
                    # store src_ap [128, nj, RE] (row r = r0 + j*128 + p) to the
                    # split tables; nrows a multiple of 128 except final block.
                    nj = (nrows + 127) // 128
                    if r0 + nrows <= LO or r0 >= LO:
                        tbl = table_lo if r0 + nrows <= LO else table_hi
                        rb = r0 if r0 + nrows <= LO else r0 - LO
                        if nrows % 128 == 0:
                            dstp = tbl[rb:rb + nrows, :].rearrange(
                                "(j p) e -> p j e", p=128)
                            nc.scalar.dma_start(dstp, src_ap[:, :nj, :])
                        else:
                            for j in range(nj):
                                mj = min(128, nrows - j * 128)
                                nc.scalar.dma_start(
                                    tbl[rb + j * 128:rb + j * 128 + mj, :],
                                    src_ap[:mj, j, :])
                        return
                    for j in range(nj):
                        mj = min(128, nrows - j * 128)
                        rj = r0 + j * 128
                        if rj + mj <= LO:
                            nc.scalar.dma_start(table_lo[rj:rj + mj, :],
                                                src_ap[:mj, j, :])
                        elif rj >= LO:
                            nc.scalar.dma_start(table_hi[rj - LO:rj - LO + mj, :],
                                                src_ap[:mj, j, :])
                        else:
                            cut = LO - rj
                            nc.scalar.dma_start(table_lo[rj:LO, :],
                                                src_ap[:cut, j, :])
                            nc.scalar.dma_start(table_hi[0:mj - cut, :],
                                                src_ap[cut:mj, j, :])

                def finalize_window(wdx):
                    dn = sp.tile([128, 4], f32, tag="dn", name=f"dn{wdx}")
                    nc.scalar.activation(dn[:], slab[:, wdx, 256:260], AF.Identity, bias=e30[:, :])
                    rd = sp.tile([128, 4], f32, tag="rd", name=f"rd{wdx}")
                    nc.vector.reciprocal(rd[:], dn[:])
                    tt = sp.tile([128, 256], bf16, tag="tt", name=f"tt{wdx}")
                    nc.vector.tensor_tensor(
                        out=tt[:].rearrange("p (h d) -> p h d", h=4),
                        in0=slab[:, wdx, :256].rearrange("p (h d) -> p h d", h=4),
                        in1=rd[:].broadcast_to([128, 4, 64]),
                        op=AL.mult)
                    ttsb = sp.tile([128, 2, 128], bf16, tag="ttsb", name=f"ttsb{wdx}")
                    yps = ypp.tile([128, D], f32, tag="yps", name=f"yps{wdx}")
                    for k in range(2):
                        ttp = tpp.tile([128, 128], bf16, tag="ttp",
                                       name=f"ttp{wdx}_{k}")
                        nc.tensor.transpose(ttp[:], tt[:, k * 128:(k + 1) * 128],
                                            ident[:])
                        nc.scalar.copy(ttsb[:, k, :], ttp[:])
                        nc.tensor.matmul(out=yps[:], lhsT=ttsb[:, k, :],
                                         rhs=wstk_sb[:, k, :],
                                         start=(k == 0), stop=(k == 1))
                    nc.vector.scalar_tensor_tensor(
                        out=y_sb[:, wdx, :], in0=yps[:], scalar=0.25, in1=bias_sb[:],
                        op0=AL.mult, op1=AL.add)
                    sq = sp.tile([128, D], f32, tag="sq", name=f"sq{wdx}")
                    nc.scalar.square(sq[:], y_sb[:, wdx, :])
                    msk = masks_sb[:, wdx:wdx + 1]
                    nc.tensor.matmul(out=bn_s[:], lhsT=msk, rhs=y_sb[:, wdx, :],
                                     start=(wdx == 0), stop=(wdx == NW - 1))
                    nc.tensor.matmul(out=bn_q[:], lhsT=msk, rhs=sq[:],
                                     start=(wdx == 0), stop=(wdx == NW - 1))

                wpt_by_sec = {}

                def emit_call(ci):
                    kind = sched["call_kind"][ci]
                    tbl = table_lo[:, :] if kind == 0 else table_hi[:, :]
                    q = ci % 4
                    live = [(b, binfo[ci * BPC + b]) for b in range(BPC)
                            if not binfo[ci * BPC + b]["dead"]]
                    if not live:
                        return
                    nb = live[-1][0] + 1  # dead batches are a strict suffix
                    nidx = nb * 128
                    gt = gp.tile([128, BPC, RE], bf16, tag="g")
                    nc.gpsimd.dma_gather(
                        out_ap=gt[:, :nb, :], in_ap=tbl,
                        idxs_ap=gidx_sb[:, ci * (CALL // 16):
                                        ci * (CALL // 16) + nidx // 16],
                        num_idxs=nidx, num_idxs_reg=nidx, elem_size=RE,
                        queue_num=q, single_packet=False)
                    indall = ip.tile([128, BPC * 256], fp8, tag="ind")
                    nc.sync.dma_start(indall[:], INDB[:, ci * BPC * 256:(ci + 1) * BPC * 256])
                    ind_t = indall[:, :BPC * 128]
                    indt_t = indall[:, BPC * 128:]

                    adst_pt = app.tile([128, BPC, 4], f32, tag="adst")
                    for b, info in live:
                        nc.tensor.matmul(
                            out=adst_pt[:, b, :],
                            lhsT=indt_t[:, b * 128:(b + 1) * 128],
                            rhs=a_dst[:, info["w"], :],
                            start=True, stop=True)
                    e0 = sp.tile([128, BPC, 4], f32, tag="e0")
                    nc.vector.tensor_tensor(
                        out=e0[:, :nb], in0=gt[:].bitcast(f32)[:, :nb, 32:36],
                        in1=adst_pt[:, :nb], op=AL.add)
                    e1 = sp.tile([128, BPC, 4], f32, tag="e1")
                    nc.vector.scalar_tensor_tensor(
                        out=e1[:, :nb], in0=e0[:, :nb], scalar=NEG, in1=e0[:, :nb],
                        op0=AL.mult, op1=AL.max)
                    msg = mp.tile([128, BPC, 260], bf16, tag="msg")
                    nc.scalar.activation(msg[:, :nb, 256:260], e1[:, :nb], AF.Exp)
                    nc.vector.tensor_tensor(
                        out=msg[:, :nb, :256].rearrange("p c (h d) -> p c h d", h=4),
                        in0=gt[:, :nb, 0:64].unsqueeze(2).broadcast_to(
                            [128, nb, 4, 64]),
                        in1=msg[:, :nb, 256:260].unsqueeze(3).broadcast_to(
                            [128, nb, 4, 64]),
                        op=AL.mult)
                    for b, info in live:
                        s = info["sec"]
                        if info["start"]:
                            wpt_by_sec[s] = wp.tile([128, 260], f32, tag="wpt", name=f"wpt{s}")
                        nc.tensor.matmul(
                            out=wpt_by_sec[s][:],
                            lhsT=ind_t[:, b * 128:(b + 1) * 128],
                            rhs=msg[:, b, :],
                            start=info["start"], stop=info["stop"])
                        if info["stop"]:
                            wdx = info["w"]
                            if info["kind"] == 0:
                                nc.scalar.copy(slab[:, wdx, :], wpt_by_sec[s][:])
                            else:
                                nc.vector.tensor_tensor(
                                    out=slab[:, wdx, :], in0=slab[:, wdx, :],
                                    in1=wpt_by_sec[s][:], op=AL.add)
                            del wpt_by_sec[s]
                            if info["kind"] == 1:
                                finalize_window(wdx)

                n_iters = (N + 1023) // 1024
                lo_iters = (LO + 1023) // 1024  # chunks covering the lo half
                lo_calls = [ci for ci in range(NCALLS) if sched["call_kind"][ci] == 0]
                hi_calls = [ci for ci in range(NCALLS) if sched["call_kind"][ci] == 1]

                with tc.tile_pool(name="pt_ps", bufs=2, space="PSUM") as tps, \
                     tc.tile_pool(name="pt_ps2", bufs=2, space="PSUM") as tps2, \
                     tc.tile_pool(name="pt_ps3", bufs=1, space="PSUM") as tps3:

                    def phase_a():
                        # a_dst for own slab (from xTs, padded to NW*128)
                        for t in range((NW * 128 + 511) // 512):
                            c0 = t * 512
                            nt = min(512, NW * 128 - c0)
                            xta = tp.tile([D, 512], bf16, tag="xta")
                            nc.sync.dma_start(xta[:, :nt], xTs[:, c0:c0 + nt])
                            ma = tps.tile([128, 512], f32, tag="m1")
                            nc.tensor.matmul(out=ma[:D, :nt], lhsT=w1t_sb[:],
                                             rhs=xta[:, :nt], start=True, stop=True)
                            x1a = tp.tile([D, 512], bf16, tag="x1a")
                            prelu_apply(x1a[:, :nt], ma[:D, :nt], D, nt, "m1ca")
                            adp = tps3.tile([128, 4, 8], f32, tag="as")
                            j = 0
                            while j * 128 < nt:
                                nc.tensor.matmul(out=adp[:, j, 0:4],
                                                 lhsT=x1a[:, j * 128:(j + 1) * 128],
                                                 rhs=cd_sb[:], start=True, stop=True)
                                j += 1
                            w0 = c0 // 128
                            nc.vector.tensor_copy(a_dst[:, w0:w0 + j, :], adp[:, :j, 0:4])

                    def chunk_body(t):
                        c0 = t * 1024
                        nt = min(1024, N - c0)  # 1024 or 848 on last
                        na = min(512, nt)
                        nb_ = nt - na
                        xt = tp.tile([128, 512], bf16, tag="xt")
                        if nb_ == 512:
                            nc.sync.dma_start(xt[:64, :], xT[:, c0:c0 + 512])
                            nc.sync.dma_start(xt[64:, :], xT[:, c0 + 512:c0 + 1024])
                        else:
                            nc.vector.memset(xt[64:, :], 0.0)
                            nc.sync.dma_start(xt[:64, :na], xT[:, c0:c0 + na])
                            if nb_ > 0:
                                nc.sync.dma_start(xt[64:, :nb_],
                                                  xT[:, c0 + 512:c0 + 512 + nb_])
                        m1 = tps.tile([128, 512], f32, tag="m1")
                        nc.tensor.matmul(out=m1[:], lhsT=w1b_sb[:], rhs=xt[:],
                                         start=True, stop=True)
                        x1 = tp.tile([128, 512], bf16, tag="x1")
                        prelu_apply(x1[:], m1[:], 128, 512, "m1c")
                        tpall = tps2.tile([128, 4, 128], bf16, tag="tp")
                        asall = tps3.tile([128, 4, 8], f32, tag="as")
                        nja = (na + 127) // 128
                        for j in range(4):
                            if j * 128 >= na and j * 128 >= nb_:
                                break
                            nc.tensor.transpose(tpall[:, j, :],
                                                x1[:, j * 128:(j + 1) * 128], ident[:])
                            nc.tensor.matmul(out=asall[:, j, :],
                                             lhsT=x1[:, j * 128:(j + 1) * 128],
                                             rhs=cs2_sb[:], start=True, stop=True)
                        rowb = tp.tile([128, 2, 4, RE], bf16, tag="rowb")
                        nc.vector.tensor_copy(rowb[:, 0, :nja, 0:64], tpall[:, :nja, 0:64])
                        nc.vector.tensor_copy(
                            rowb[:].bitcast(f32)[:, 0, :nja, 32:36],
                            asall[:, :nja, 0:4])
                        if nb_ > 0:
                            njb = (nb_ + 127) // 128
                            nc.vector.tensor_copy(rowb[:, 1, :njb, 0:64],
                                                  tpall[:, :njb, 64:128])
                            nc.vector.tensor_copy(
                                rowb[:].bitcast(f32)[:, 1, :njb, 32:36],
                                asall[:, :njb, 4:8])
                        if nt == 1024 and (c0 + 1024 <= LO or c0 >= LO):
                            store_span(c0, 1024,
                                       rowb[:].rearrange("p k j e -> p (k j) e"))
                        else:
                            store_span(c0, na, rowb[:, 0, :, :])
                            if nb_ > 0:
                                store_span(c0 + 512, nb_, rowb[:, 1, :, :])

                    # pad rows first
                    padrow = tp.tile([1, RE], bf16, tag="pad")
                    nc.vector.memset(padrow[:], 0.0)
                    nc.vector.memset(padrow[:].bitcast(f32)[:, 32:36], -1e4)
                    nc.sync.dma_start(table_lo[LO:LO + 1, :], padrow[:])
                    nc.sync.dma_start(table_hi[LO:LO + 1, :], padrow[:])

                    phase_a()
                    for t in range(lo_iters):
                        chunk_body(t)
                    # lo gathers start here (all table_lo writes are emitted);
                    # hi chunks stream concurrently on the compute engines.
                    hi_chunks = list(range(lo_iters, n_iters))
                    nh = 0
                    for i, ci in enumerate(lo_calls):
                        emit_call(ci)
                        if i % 2 == 0 and nh < len(hi_chunks):
                            chunk_body(hi_chunks[nh])
                            nh += 1
                    for t in hi_chunks[nh:]:
                        chunk_body(t)

                with tc.tile_pool(name="pe_tp", bufs=1, space="PSUM") as tpp, \
                     tc.tile_pool(name="pe_yp", bufs=1, space="PSUM") as ypp, \
                     tc.tile_pool(name="pf_ps", bufs=1, space="PSUM") as fps:
                    bn_s = fps.tile([1, D], f32, tag="bns")
                    bn_q = fps.tile([1, D], f32, tag="bnq")

                    for ci in hi_calls:
                        emit_call(ci)

                    # ---------------- phase B: BN + relu + store ---------------
                    fp_ = sp
                    st = fp_.tile([1, 128], f32, tag="st")
                    nc.vector.tensor_copy(st[:, :64], bn_s[:])
                    nc.vector.tensor_copy(st[:, 64:], bn_q[:])
                    nc.sync.dma_start(cc_in[:], st[:])
                    nc.gpsimd.collective_compute(
                        "AllReduce", AL.add, replica_groups=[list(range(NC))],
                        ins=[cc_in[:].opt()], outs=[cc_out[:].opt()])
                    st2 = fp_.tile([1, 128], f32, tag="st2")
                    nc.sync.dma_start(st2[:], cc_out[:])
                    mean = fp_.tile([1, D], f32, tag="mean")
                    nc.scalar.mul(mean[:], st2[:, :64], 1.0 / N)
                    ex2 = fp_.tile([1, D], f32, tag="ex2")
                    nc.scalar.mul(ex2[:], st2[:, 64:], 1.0 / N)
                    msq = fp_.tile([1, D], f32, tag="msq")
                    nc.scalar.square(msq[:], mean[:])
                    var = fp_.tile([1, D], f32, tag="var")
                    nc.vector.tensor_tensor(out=var[:], in0=ex2[:], in1=msq[:],
                                            op=AL.subtract)
                    sd = fp_.tile([1, D], f32, tag="sd")
                    nc.scalar.activation(sd[:], var[:], AF.Sqrt, bias=ebn[:, :])
                    rs = fp_.tile([1, D], f32, tag="rs")
                    nc.vector.reciprocal(rs[:], sd[:])
                    scsh = fp_.tile([1, 128], f32, tag="scsh")
                    nc.vector.tensor_tensor(out=scsh[:, :64], in0=gb_sb[:, :64], in1=rs[:],
                                            op=AL.mult)
                    mssc = fp_.tile([1, D], f32, tag="mssc")
                    nc.vector.tensor_tensor(out=mssc[:], in0=mean[:], in1=scsh[:, :64],
                                            op=AL.mult)
                    nc.vector.tensor_tensor(out=scsh[:, 64:], in0=gb_sb[:, 64:], in1=mssc[:],
                                            op=AL.subtract)
                    bc = sp.tile([128, 128], f32, tag="bc")
                    nc.gpsimd.partition_broadcast(bc[:], scsh[:])
                    z = zp.tile([128, NW, D], f32, tag="za")
                    nc.vector.tensor_tensor(
                        out=z[:],
                        in0=y_sb[:], in1=bc[:, :64].unsqueeze(1).broadcast_to(
                            [128, NW, 64]),
                        op=AL.mult)
                    z2 = zp.tile([128, NW, D], f32, tag="zb")
                    nc.vector.tensor_tensor(
                        out=z2[:], in0=z[:], in1=bc[:, 64:].unsqueeze(1).broadcast_to(
                            [128, NW, 64]),
                        op=AL.add)
                    zo = zp.tile([128, NW, D], f32, tag="za", name="zo")
                    nc.scalar.activation(zo[:], z2[:], AF.Relu)
                    nc.sync.dma_start(
                        out_slab[:, :].rearrange("(w p) d -> p w d", p=128),
                        zo[:, :, :])

    nc.compile()
    return nc


def kernel(x, edge_index, W_lin, b_lin, prelu_w, W_gat, att_src, att_dst,
           gat_bias, bn_gamma, bn_beta):
    global LAST_EXEC_NS, LAST_TRACE
    from concourse import bass_utils

    x = np.asarray(x, np.float32)
    edge_index = np.asarray(edge_index)
    W_lin = np.asarray(W_lin, np.float32)
    b_lin = np.asarray(b_lin, np.float32)
    prelu_w = np.asarray(prelu_w, np.float32)
    W_gat = np.asarray(W_gat, np.float32)
    att_src = np.asarray(att_src, np.float32)
    att_dst = np.asarray(att_dst, np.float32)
    gat_bias = np.asarray(gat_bias, np.float32)
    bn_gamma = np.asarray(bn_gamma, np.float32)
    bn_beta = np.asarray(bn_beta, np.float32)

    # b_lin is zero in the reference setup; if nonzero, do the pre-linear
    # exactly on host and feed the device an identity pre-stage.
    if np.any(b_lin != 0):
        x1_host = x @ W_lin.T + b_lin
        x1_host = np.where(x1_host >= 0, x1_host, prelu_w * x1_host)
        xT_eff = np.ascontiguousarray(x1_host.T)
        W1_eff = np.eye(64, dtype=np.float32)
        prelu_eff = np.ones((64,), np.float32)
    else:
        xT_eff = np.ascontiguousarray(x.T)
        W1_eff = W_lin
        prelu_eff = prelu_w

    prelu_uniform = bool(np.all(prelu_eff == prelu_eff[0]))
    prelu_value = float(prelu_eff[0]) if prelu_uniform else 0.0

    key = (hashlib.sha1(np.ascontiguousarray(edge_index).tobytes()).hexdigest(),
           prelu_uniform, prelu_value)
    if key not in _CACHE:
        sched, blobs = _schedule_and_blobs(edge_index)
        nc = _build_program(sched, prelu_uniform, prelu_value)
        _CACHE[key] = (sched, blobs, nc)
    sched, blobs, nc = _CACHE[key]

    C_src = np.zeros((64, 4), np.float32)
    C_dst = np.zeros((64, 4), np.float32)
    for h in range(H):
        Wh = W_gat[h * 64:(h + 1) * 64, :]  # [64, 64] maps x1 -> head h
        C_src[:, h] = Wh.T @ att_src[h]
        C_dst[:, h] = Wh.T @ att_dst[h]

    bf = ml_dtypes.bfloat16
    W1T_np = np.ascontiguousarray(W1_eff.T).astype(bf)  # [din, dout]
    W1B_np = np.zeros((128, 128), np.float32)
    W1B_np[:64, :64] = W1_eff.T
    W1B_np[64:, 64:] = W1_eff.T
    CS2_np = np.zeros((128, 8), np.float32)
    CS2_np[:64, 0:4] = C_src
    CS2_np[64:, 4:8] = C_src
    # WSTK[k*128+p, d'] laid out [128, 2*64]: row p, block k: W_h.T stacked
    # rows hd = h*64+dk -> Wstk[h*64+dk, d'] = W_gat[h*64+d', dk]
    WSTK_np = np.zeros((256, 64), np.float32)
    for h in range(H):
        WSTK_np[h * 64:(h + 1) * 64, :] = W_gat[h * 64:(h + 1) * 64, :].T
    WSTK_2 = np.concatenate([WSTK_np[:128], WSTK_np[128:]], axis=1)  # [128, 128]

    prelu2_np = np.concatenate([prelu_eff, prelu_eff]).reshape(128, 1)
    xT_bf = xT_eff.astype(bf)

    perm_core = sched["perm_core"]
    perm_slot = sched["perm_slot"]
    in_maps = []
    for c in range(NC):
        xs = np.zeros((64, NW * 128), np.float32)
        own = np.nonzero(perm_core == c)[0]
        xs[:, perm_slot[own]] = xT_eff[:, own]
        msks = np.zeros((NW * 128,), np.float32)
        msks[perm_slot[own]] = 1.0
        msks = np.ascontiguousarray(msks.reshape(NW, 128).T)
        in_maps.append(dict(
            xT=xT_bf,
            xTs=xs.astype(bf),
            W1B=W1B_np.astype(bf),
            W1T=W1T_np,
            CS2=CS2_np.astype(bf),
            CD=C_dst.astype(bf),
            WSTK=WSTK_2.astype(bf),
            prelu2=prelu2_np,
            prelu1=prelu_eff.reshape(64, 1),
            GIDX=blobs[c]["GIDX"], INDB=blobs[c]["INDB"],
            bias128=np.tile(gat_bias[None, :], (128, 1)),
            MASKS=msks,
            gb_row=np.concatenate([bn_gamma, bn_beta])[None, :],
        ))

    trace = os.environ.get("GAT_TRACE", "0") == "1"
    if trace:
        _install_ntff_shim()
    res = bass_utils.run_bass_kernel_spmd(nc, in_maps, core_ids=list(range(NC)),
                                          trace=trace)
    LAST_EXEC_NS = res.exec_time_ns
    LAST_TRACE = res.instructions_and_trace
    out = np.empty((N, D), np.float32)
    for c in range(NC):
        own = np.nonzero(perm_core == c)[0]
        out[own] = res.results[c]["out_slab"][perm_slot[own]]
    return out

